# revision 8
# baseline (speedup 1.0000x reference)
"""GNN attention layer (N=50000 nodes, K=15 neighbors, H=128) on 8 TRN2 cores.

Math (reference):
    nbr = dst.reshape(N, K)
    q  = x @ Wq.T + bq                      # [N, 64]
    kf = x[nbr] @ Wk.T + bk                 # [N, K, 64]
    scores = (q . kf) / scale               # [N, K]
    attn = softmax(scores * (nbr != 0))     # [N, K]
    cagg = sum_k attn * [x[nbr], pts[nbr] - pts]   # [N, 131]
    out  = pts + (cagg @ Wc.T + bc) @ Wo.T + bo

Algebraic restructuring (exact up to fp assoc):
  * scores[i,k] = u[i] . x[nbr[i,k]] + s[i], with
        u = (x @ (Wq.T @ Wk) + bq @ Wk) / scale      # [N, 128]
        s = (x @ (Wq.T @ bk) + bq.bk) / scale        # [N]
    (s must be added before the mask multiply).
  * since sum_k attn = 1:
        disp = Wf @ cagg + bf,  Wf = Wo @ Wc [3,131], bf = Wo @ bc + bo
        cagg_p = (sum_k attn * pts[nbr]) - pts
    The -pts and +pts(residual) terms are folded into an extended matvec:
        out[p,c] = sum_f Wfx[c,f]*caggx[p,f] + bf[c]
    where caggx = [sum_k attn*tabrow(132) | pts(3)] (PSUM, PE-accumulated)
    and Wfx[c,:] = [Wf[c,0:131] | 0 | I3[c,:] - Wf[c,128:131]].

Gather: this image has no GPSIMD extended-instruction ucode (bedrock), and
the indirect1d ucode only supports ONE int32 index per partition (multi-
index offset APs degenerate to a single lane; HW-probed), so each gather
instruction fetches 128 rows from a fused bf16 table [x(128)|pts(3)|pad]
(528B... 264B rows).  A 128-node tile needs K=15 gathers.  Descriptor
generation serializes on the Pool engine (~1.1us/instr) — that is the hard
floor.  Everything else is kept OFF the DVE<->GpSimd shared SBUF port pair
(an exclusive, full-instruction lock that stalls SWDGE desc-gen):
  * DVE two-tensor ops read one operand from PSUM (u, caggx) — no shared
    port use.
  * The attention-weighted aggregation runs on PE as 15 PSUM-accumulated
    diag(attn_k) @ g_k matmuls; diag matrices are built by the Scalar
    engine (per-partition scale of a cached identity).
  * softmax scaling and small copies run on the Scalar engine.
Gathers are also spread over 4 SWDGE queues (k%4) for ring headroom.

Sharding: nodes split contiguously over 8 cores (6250 each, padded to
6272 = 49*128); the gather table is replicated per core. No collectives.
"""

import numpy as np

N = 50000
K = 15
H = 128
NCORES = 8
SH = N // NCORES          # 6250 real nodes per core
P = 128
NT = 49                   # tiles per core
NP = NT * P               # 6272 padded nodes per core
TW = H + 4                # table row width [x 128 | pts 3 | pad]
SCALE = float(np.sqrt(64.0) + 1e-6)

_NC_CACHE = {}


def build_nc():
    import contextlib

    import concourse.bacc as bacc
    import concourse.bass as bass
    import concourse.mybir as mybir
    import concourse.tile as tile
    from concourse.masks import make_identity

    f32 = mybir.dt.float32
    bf16 = mybir.dt.bfloat16
    i32 = mybir.dt.int32
    Alu = mybir.AluOpType
    Act = mybir.ActivationFunctionType

    nc = bacc.Bacc("TRN2", target_bir_lowering=False, debug=False,
                   num_devices=NCORES, dynamic_dma_scratch_size=65536,
                   num_swdge_queues=4)

    tab_d = nc.dram_tensor("tab", [N, TW], bf16, kind="ExternalInput")
    xT_d = nc.dram_tensor("xT", [P, NP], f32, kind="ExternalInput")
    pts_d = nc.dram_tensor("pts", [NP, 4], f32, kind="ExternalInput")
    idx_d = nc.dram_tensor("idx", [P, NT * K], i32, kind="ExternalInput")
    Wq_d = nc.dram_tensor("Wq", [64, H], f32, kind="ExternalInput")
    Wk_d = nc.dram_tensor("Wk", [64, H], f32, kind="ExternalInput")
    bq_d = nc.dram_tensor("bq", [64, 1], f32, kind="ExternalInput")
    bk_d = nc.dram_tensor("bk", [64, 1], f32, kind="ExternalInput")
    Wc_d = nc.dram_tensor("Wc", [131, 131], f32, kind="ExternalInput")
    bc_d = nc.dram_tensor("bc", [131, 1], f32, kind="ExternalInput")
    Wo_d = nc.dram_tensor("Wo", [3, 131], f32, kind="ExternalInput")
    bo_d = nc.dram_tensor("bo", [1, 3], f32, kind="ExternalInput")
    out_d = nc.dram_tensor("out", [NP, 4], f32, kind="ExternalOutput")

    with tile.TileContext(nc) as tc, contextlib.ExitStack() as ctx:
        const = ctx.enter_context(tc.tile_pool(name="const", bufs=1))

        ones1 = const.tile([1, P], f32)
        nc.vector.memset(ones1[:], 1.0)

        idn = const.tile([P, P], f32)
        make_identity(nc, idn[:])

        # all gather indices, preloaded once: column t*K+k holds nbr[t*128+p, k]
        idx_all = const.tile([P, NT * K], i32)
        nc.sync.dma_start(out=idx_all[:], in_=idx_d.ap())
        # mask = (nbr != 0) == min(idx, 1) for idx >= 0, built once for all
        # tiles (keeps per-tile DVE work off the shared SBUF port).
        idxf_all = const.tile([P, NT * K], f32)
        nc.vector.tensor_copy(out=idxf_all[:], in_=idx_all[:])
        mask_all = const.tile([P, NT * K], f32)
        nc.vector.tensor_scalar(out=mask_all[:], in0=idxf_all[:], scalar1=1.0,
                                scalar2=None, op0=Alu.min)

        # ---------- one-time weight prep ----------
        with tc.tile_pool(name="wprep", bufs=1) as wp:
            Wq_s = wp.tile([64, H], f32)
            nc.scalar.dma_start(out=Wq_s[:], in_=Wq_d.ap())
            Wkx_s = wp.tile([64, H + 1], f32)
            nc.scalar.dma_start(out=Wkx_s[:, 0:H], in_=Wk_d.ap())
            nc.scalar.dma_start(out=Wkx_s[:, H:H + 1], in_=bk_d.ap())
            bq_s = wp.tile([64, 1], f32)
            nc.scalar.dma_start(out=bq_s[:], in_=bq_d.ap())
            Wo_s = wp.tile([3, 131], f32)
            nc.scalar.dma_start(out=Wo_s[:], in_=Wo_d.ap())
            Wc0_s = wp.tile([P, 131], f32)
            nc.scalar.dma_start(out=Wc0_s[:], in_=Wc_d.ap()[0:P, :])
            Wc1_s = wp.tile([3, 131], f32)
            nc.scalar.dma_start(out=Wc1_s[:], in_=Wc_d.ap()[P:131, :])
            bc0_s = wp.tile([P, 1], f32)
            nc.scalar.dma_start(out=bc0_s[:], in_=bc_d.ap()[0:P, :])
            bc1_s = wp.tile([3, 1], f32)
            nc.scalar.dma_start(out=bc1_s[:], in_=bc_d.ap()[P:131, :])
            bo_s = wp.tile([1, 3], f32)
            nc.scalar.dma_start(out=bo_s[:], in_=bo_d.ap())

            with tc.tile_pool(name="wprep_psA", bufs=1, space="PSUM") as wpp:
                woT0_p = wpp.tile([P, 3], f32, space="PSUM")
                nc.tensor.transpose(out=woT0_p[:], in_=Wo_s[:, 0:P],
                                    identity=idn[0:3, 0:3])
                woT0_s = wp.tile([P, 3], f32)
                nc.vector.tensor_copy(out=woT0_s[:], in_=woT0_p[:])
                woT1_p = wpp.tile([3, 3], f32, space="PSUM")
                nc.tensor.transpose(out=woT1_p[:], in_=Wo_s[:, P:131],
                                    identity=idn[0:3, 0:3])
                woT1_s = wp.tile([3, 3], f32)
                nc.vector.tensor_copy(out=woT1_s[:], in_=woT1_p[:])

                # Wf = Wo @ Wc  [3,131]
                Wf_p = wpp.tile([3, 131], f32, space="PSUM")
                nc.tensor.matmul(out=Wf_p[:], lhsT=woT0_s[:], rhs=Wc0_s[:],
                                 start=True, stop=False)
                nc.tensor.matmul(out=Wf_p[:], lhsT=woT1_s[:], rhs=Wc1_s[:],
                                 start=False, stop=True)
                Wf_s = wp.tile([3, 131], f32)
                nc.vector.tensor_copy(out=Wf_s[:], in_=Wf_p[:])

                # bfT = (Wo @ bc).T [1,3] ; + bo
                bfT_p = wpp.tile([1, 3], f32, space="PSUM")
                nc.tensor.matmul(out=bfT_p[:], lhsT=bc0_s[:], rhs=woT0_s[:],
                                 start=True, stop=False)
                nc.tensor.matmul(out=bfT_p[:], lhsT=bc1_s[:], rhs=woT1_s[:],
                                 start=False, stop=True)
                bfT_s = wp.tile([1, 3], f32)
                nc.vector.tensor_add(out=bfT_s[:], in0=bfT_p[:], in1=bo_s[:])

            with tc.tile_pool(name="wprep_psB", bufs=1, space="PSUM") as wpp2:
                # wfx rows: [Wf[c,0:131] | 0 | I3[c,:]-Wf[c,128:131] | bf[c]],
                # replicated across partitions (stride-0 partition APs are
                # illegal on DVE): e_c row extract + ones-outer-product.
                # The bias column pairs with caggx[:,135] = sum(exp), so the
                # whole output is one matvec scaled by 1/sum(exp) at the end.
                wfx_s = const.tile([P, 3, 136], f32)
                nc.vector.memset(wfx_s[:], 0.0)
                for c in range(3):
                    row_p = wpp2.tile([1, 131], f32, space="PSUM",
                                      name="row_p")
                    nc.tensor.matmul(out=row_p[:], lhsT=idn[0:3, c:c + 1],
                                     rhs=Wf_s[:], start=True, stop=True)
                    row_s = wp.tile([1, 136], f32, name=f"row_s{c}")
                    nc.vector.memset(row_s[:], 0.0)
                    nc.scalar.activation(out=row_s[:, 0:131], in_=row_p[:],
                                         func=Act.Copy)
                    # cols 132..134: I3[c,:] - Wf[c,128:131]
                    rowI = wp.tile([1, 3], f32, name=f"rowI{c}")
                    nc.vector.memset(rowI[:], 0.0)
                    nc.vector.memset(rowI[:, c:c + 1], 1.0)
                    nc.vector.tensor_sub(out=row_s[:, 132:135],
                                         in0=rowI[:],
                                         in1=row_s[:, 128:131])
                    # col 135: bf[c]
                    nc.scalar.activation(out=row_s[:, 135:136],
                                         in_=bfT_s[:, c:c + 1],
                                         func=Act.Copy)
                    wfx_p = wpp2.tile([P, 136], f32, space="PSUM",
                                      name="wfx_p")
                    nc.tensor.matmul(out=wfx_p[:], lhsT=ones1[:],
                                     rhs=row_s[:], start=True, stop=True)
                    nc.scalar.activation(out=wfx_s[:, c, :], in_=wfx_p[:],
                                         func=Act.Copy)

                # M_ext = [Wq.T @ Wk | Wq.T @ bk] / scale  [128, 129]
                Mw_p = wpp2.tile([P, H + 1], f32, space="PSUM")
                nc.tensor.matmul(out=Mw_p[:], lhsT=Wq_s[:], rhs=Wkx_s[:],
                                 start=True, stop=True)
                Mx_s = const.tile([P, H + 1], f32)
                nc.scalar.activation(out=Mx_s[:], in_=Mw_p[:], func=Act.Copy,
                                     scale=1.0 / SCALE)

                # [c1 | s2] = [bq @ Wk | bq.bk] / scale  [1, 129]
                cs_p = wpp2.tile([1, H + 1], f32, space="PSUM")
                nc.tensor.matmul(out=cs_p[:], lhsT=bq_s[:], rhs=Wkx_s[:],
                                 start=True, stop=True)
                cs_s = const.tile([1, H + 1], f32)
                nc.scalar.activation(out=cs_s[:], in_=cs_p[:], func=Act.Copy,
                                     scale=1.0 / SCALE)

        # ---------- main loop ----------
        sb = ctx.enter_context(tc.tile_pool(name="sb", bufs=4))
        gp = ctx.enter_context(tc.tile_pool(name="gp", bufs=8))
        big = ctx.enter_context(tc.tile_pool(name="big", bufs=4))
        dg = ctx.enter_context(tc.tile_pool(name="dg", bufs=4))
        pp = ctx.enter_context(tc.tile_pool(name="pp", bufs=4, space="PSUM"))

        for t in range(NT):
            rows = slice(t * P, (t + 1) * P)

            xT_t = sb.tile([P, P], f32)
            nc.sync.dma_start(out=xT_t[:], in_=xT_d.ap()[:, rows])
            idx_t = idx_all[:, t * K:(t + 1) * K]
            pts_t = sb.tile([P, 4], f32)
            nc.sync.dma_start(out=pts_t[:], in_=pts_d.ap()[rows, :])

            # K single-index-per-partition indirect gathers (128 rows each),
            # spread over the 4 SWDGE queues for descriptor-ring headroom.
            g_t = gp.tile([P, K, TW], bf16)
            for k in range(K):
                inst = nc.gpsimd.indirect_dma_start(
                    out=g_t[:, k, :],
                    out_offset=None,
                    in_=tab_d.ap(),
                    in_offset=bass.IndirectOffsetOnAxis(
                        ap=idx_t[:, k:k + 1], axis=0),
                )
                qn = k % 4
                if qn:
                    inst.ins.queue = f"qPoolDynamic{qn}"

            # u_ext = xT.T @ M_ext + bcast([c1|s2])  ->  [p, 129] = [u | s]
            # (kept in PSUM: DVE reads of it avoid the shared SBUF port)
            u_p = pp.tile([P, H + 1], f32, space="PSUM")
            nc.tensor.matmul(out=u_p[:], lhsT=xT_t[:], rhs=Mx_s[:],
                             start=True, stop=False)
            nc.tensor.matmul(out=u_p[:], lhsT=ones1[:], rhs=cs_s[:],
                             start=False, stop=True)
            s_f = sb.tile([P, 1], f32)
            nc.scalar.activation(out=s_f[:], in_=u_p[:, H:H + 1],
                                 func=Act.Copy)

            # scores: raw[p,k] = sum_h u[p,h] * g[p,k,h]  (+ s, * mask)
            # (split so only the last 3 slots' product trails the final
            # gather — shortens the per-tile critical chain)
            prod = big.tile([P, K, H], bf16)
            nc.vector.tensor_mul(
                out=prod[:, 0:12, :],
                in0=g_t[:][:, 0:12, 0:H],
                in1=u_p[:, 0:H].unsqueeze(1).to_broadcast([P, 12, H]),
            )
            nc.vector.tensor_mul(
                out=prod[:, 12:K, :],
                in0=g_t[:][:, 12:K, 0:H],
                in1=u_p[:, 0:H].unsqueeze(1).to_broadcast([P, K - 12, H]),
            )
            raw = sb.tile([P, K], f32)
            nc.vector.tensor_reduce(out=raw[:, 0:12], in_=prod[:, 0:12, :],
                                    axis=mybir.AxisListType.X, op=Alu.add)
            nc.vector.tensor_reduce(out=raw[:, 12:K], in_=prod[:, 12:K, :],
                                    axis=mybir.AxisListType.X, op=Alu.add)
            sc = sb.tile([P, K], f32)
            nc.vector.tensor_scalar(out=sc[:], in0=raw[:],
                                    scalar1=s_f[:], scalar2=None,
                                    op0=Alu.add)
            scm = sb.tile([P, K], f32)
            nc.vector.tensor_mul(out=scm[:], in0=sc[:],
                                 in1=mask_all[:, t * K:(t + 1) * K])

            # softmax over k (scores are bounded |.|<~8, skip max-sub).
            # The normalization 1/sum(exp) is applied once at the very end,
            # so the aggregation uses raw exp weights and the reciprocal
            # stays off the critical chain.
            e_t = sb.tile([P, K], f32)
            se_t = sb.tile([P, 1], f32)
            nc.scalar.activation(out=e_t[:], in_=scm[:], func=Act.Exp,
                                 bias=0.0, scale=1.0, accum_out=se_t[:])
            r_t = sb.tile([P, 1], f32)
            nc.vector.reciprocal(out=r_t[:], in_=se_t[:])

            # caggx = [sum_k e_k * g_k (132) | pts*S (3) | S],  S = sum(exp),
            # accumulated on PE: caggx += diag(e_k) @ g_k.  diag built on
            # the Scalar engine (per-partition scale of identity).
            caggx_p = pp.tile([P, 136], f32, space="PSUM")
            for k in range(K):
                dg_t = dg.tile([P, P], bf16)
                nc.scalar.activation(out=dg_t[:], in_=idn[:], func=Act.Copy,
                                     scale=e_t[:, k:k + 1])
                nc.tensor.matmul(out=caggx_p[:, 0:132], lhsT=dg_t[:],
                                 rhs=g_t[:, k, :], start=(k == 0),
                                 stop=(k == K - 1))
            nc.scalar.activation(out=caggx_p[:, 132:135], in_=pts_t[:, 0:3],
                                 func=Act.Copy, scale=se_t[:])
            nc.scalar.activation(out=caggx_p[:, 135:136], in_=se_t[:],
                                 func=Act.Copy)

            # disp[p,c] = Wfx[c,:] . caggx[p,:]  (PSUM-side mul avoids the
            # DVE<->GpSimd shared SBUF port); out = disp / S
            disp = sb.tile([P, 3], f32)
            junk = sb.tile([P, 136], f32)
            for c in range(3):
                nc.vector.tensor_mul(out=junk[:], in0=caggx_p[:, 0:136],
                                     in1=wfx_s[:, c, :])
                nc.vector.tensor_reduce(out=disp[:, c:c + 1], in_=junk[:],
                                        axis=mybir.AxisListType.X,
                                        op=Alu.add)

            out_t = sb.tile([P, 4], f32)
            nc.scalar.activation(out=out_t[:, 0:3], in_=disp[:],
                                 func=Act.Copy, scale=r_t[:])
            nc.sync.dma_start(out=out_d.ap()[rows, 0:3], in_=out_t[:, 0:3])

    nc.compile()
    return nc


def get_nc():
    if "nc" not in _NC_CACHE:
        _NC_CACHE["nc"] = build_nc()
    return _NC_CACHE["nc"]


def make_in_maps(sampled_points, sampled_x, Wq, bq, Wk, bk, Wc, bc, Wo, bo,
                 edge_index_filtered):
    sampled_points = np.ascontiguousarray(sampled_points, np.float32)
    sampled_x = np.ascontiguousarray(sampled_x, np.float32)
    nbr = np.ascontiguousarray(
        np.asarray(edge_index_filtered)[1].reshape(N, K)).astype(np.int32)

    import ml_dtypes
    tab = np.zeros((N, TW), ml_dtypes.bfloat16)
    tab[:, :H] = sampled_x.astype(ml_dtypes.bfloat16)
    tab[:, H:H + 3] = sampled_points.astype(ml_dtypes.bfloat16)

    shared = {
        "tab": tab,
        "Wq": np.ascontiguousarray(Wq, np.float32),
        "Wk": np.ascontiguousarray(Wk, np.float32),
        "bq": np.ascontiguousarray(np.reshape(bq, (64, 1)), np.float32),
        "bk": np.ascontiguousarray(np.reshape(bk, (64, 1)), np.float32),
        "Wc": np.ascontiguousarray(Wc, np.float32),
        "bc": np.ascontiguousarray(np.reshape(bc, (131, 1)), np.float32),
        "Wo": np.ascontiguousarray(Wo, np.float32),
        "bo": np.ascontiguousarray(np.reshape(bo, (1, 3)), np.float32),
    }

    in_maps = []
    for c in range(NCORES):
        rows = slice(c * SH, (c + 1) * SH)
        xT = np.zeros((P, NP), np.float32)
        xT[:, :SH] = sampled_x[rows].T
        pts4 = np.zeros((NP, 4), np.float32)
        pts4[:SH, :3] = sampled_points[rows]
        nb = np.zeros((NP, K), np.int32)
        nb[:SH] = nbr[rows]
        # [P, NT*K]: column t*K+k = nbr[t*128+p, k]
        idx = np.ascontiguousarray(
            nb.reshape(NT, P, K).transpose(1, 0, 2).reshape(P, NT * K))
        in_maps.append({**shared, "xT": xT, "pts": pts4, "idx": idx})
    return in_maps


def unshard(results):
    out = np.concatenate(
        [results[c]["out"][:SH, :3] for c in range(NCORES)], axis=0)
    return np.ascontiguousarray(out)


def kernel(**inputs):
    from concourse.bass_utils import run_bass_kernel_spmd

    nc = get_nc()
    in_maps = make_in_maps(**inputs)
    res = run_bass_kernel_spmd(nc, in_maps, core_ids=list(range(NCORES)))
    return unshard(res.results)


# revision 14
# speedup vs baseline: 1.1772x; 1.1772x over previous
"""GNN attention layer (N=50000 nodes, K=15 neighbors, H=128) on 8 TRN2 cores.

Math (reference):
    nbr = dst.reshape(N, K)
    q  = x @ Wq.T + bq                      # [N, 64]
    kf = x[nbr] @ Wk.T + bk                 # [N, K, 64]
    scores = (q . kf) / scale               # [N, K]
    attn = softmax(scores * (nbr != 0))     # [N, K]
    cagg = sum_k attn * [x[nbr], pts[nbr] - pts]   # [N, 131]
    out  = pts + (cagg @ Wc.T + bc) @ Wo.T + bo

Algebraic restructuring (exact up to fp assoc):
  * scores[i,k] = u[i] . x[nbr[i,k]] + s[i], with
        u = (x @ (Wq.T @ Wk) + bq @ Wk) / scale      # [N, 128]
        s = (x @ (Wq.T @ bk) + bq.bk) / scale        # [N]
    (s must be added before the mask multiply).
  * since sum_k attn = 1:
        disp = Wf @ cagg + bf,  Wf = Wo @ Wc [3,131], bf = Wo @ bc + bo
        cagg_p = (sum_k attn * pts[nbr]) - pts
    The -pts and +pts(residual) terms are folded into an extended matvec:
        out[p,c] = sum_f Wfx[c,f]*caggx[p,f] + bf[c]
    where caggx = [sum_k attn*tabrow(132) | pts(3)] (PSUM, PE-accumulated)
    and Wfx[c,:] = [Wf[c,0:131] | 0 | I3[c,:] - Wf[c,128:131]].

Gather: this image has no GPSIMD extended-instruction ucode (bedrock), and
the indirect1d ucode only supports ONE int32 index per partition (multi-
index offset APs degenerate to a single lane; HW-probed), so each gather
instruction fetches 128 rows from a fused bf16 table [x(128)|pts(3)|pad]
(528B... 264B rows).  A 128-node tile needs K=15 gathers.  Descriptor
generation serializes on the Pool engine (~1.1us/instr) — that is the hard
floor.  Everything else is kept OFF the DVE<->GpSimd shared SBUF port pair
(an exclusive, full-instruction lock that stalls SWDGE desc-gen):
  * DVE two-tensor ops read one operand from PSUM (u, caggx) — no shared
    port use.
  * The attention-weighted aggregation runs on PE as 15 PSUM-accumulated
    diag(attn_k) @ g_k matmuls; diag matrices are built by the Scalar
    engine (per-partition scale of a cached identity).
  * softmax scaling and small copies run on the Scalar engine.

Sharding: nodes split contiguously over 8 cores (6250 each, padded to
6272 = 49*128); the gather table is replicated per core. No collectives.
"""

import numpy as np

N = 50000
K = 15
H = 128
NCORES = 8
SH = N // NCORES          # 6250 real nodes per core
P = 128
NT = 49                   # tiles per core
NP = NT * P               # 6272 padded nodes per core
TW = H + 4                # table row width [x 128 | pts 3 | pad]
SCALE = float(np.sqrt(64.0) + 1e-6)

_NC_CACHE = {}


def build_nc():
    import contextlib

    import concourse.bacc as bacc
    import concourse.bass as bass
    import concourse.mybir as mybir
    import concourse.tile as tile

    f32 = mybir.dt.float32
    bf16 = mybir.dt.bfloat16
    i32 = mybir.dt.int32
    Alu = mybir.AluOpType
    Act = mybir.ActivationFunctionType

    nc = bacc.Bacc("TRN2", target_bir_lowering=False, debug=False,
                   num_devices=NCORES, dynamic_dma_scratch_size=65536,
                   num_swdge_queues=1)

    tab_d = nc.dram_tensor("tab", [N, TW], bf16, kind="ExternalInput")
    xT_d = nc.dram_tensor("xT", [P, NP], f32, kind="ExternalInput")
    pts_d = nc.dram_tensor("pts", [NP, 4], f32, kind="ExternalInput")
    idx_d = nc.dram_tensor("idx", [P, NT * K], i32, kind="ExternalInput")
    Wq_d = nc.dram_tensor("Wq", [64, H], f32, kind="ExternalInput")
    Wk_d = nc.dram_tensor("Wk", [64, H], f32, kind="ExternalInput")
    bq_d = nc.dram_tensor("bq", [64, 1], f32, kind="ExternalInput")
    bk_d = nc.dram_tensor("bk", [64, 1], f32, kind="ExternalInput")
    Wc_d = nc.dram_tensor("Wc", [131, 131], f32, kind="ExternalInput")
    bc_d = nc.dram_tensor("bc", [131, 1], f32, kind="ExternalInput")
    Wo_d = nc.dram_tensor("Wo", [3, 131], f32, kind="ExternalInput")
    bo_d = nc.dram_tensor("bo", [1, 3], f32, kind="ExternalInput")
    idn_d = nc.dram_tensor("idn", [P, P], f32, kind="ExternalInput")
    out_d = nc.dram_tensor("out", [NP, 4], f32, kind="ExternalOutput")

    with tile.TileContext(nc) as tc, contextlib.ExitStack() as ctx:
        const = ctx.enter_context(tc.tile_pool(name="const", bufs=1))

        ones1 = const.tile([1, P], f32)
        nc.vector.memset(ones1[:], 1.0)

        # identity shipped as an input (keeps make_identity's gpsimd ops off
        # the Pool engine ahead of the first gather)
        idn = const.tile([P, P], f32)
        nc.scalar.dma_start(out=idn[:], in_=idn_d.ap())

        # all gather indices, preloaded once: column t*K+k holds nbr[t*128+p, k]
        idx_all = const.tile([P, NT * K], i32)
        nc.sync.dma_start(out=idx_all[:], in_=idx_d.ap())
        # mask = (nbr != 0) == min(idx, 1) for idx >= 0, built once for all
        # tiles (keeps per-tile DVE work off the shared SBUF port).
        idxf_all = const.tile([P, NT * K], f32)
        nc.vector.tensor_copy(out=idxf_all[:], in_=idx_all[:])
        mask_all = const.tile([P, NT * K], f32)
        nc.vector.tensor_scalar(out=mask_all[:], in0=idxf_all[:], scalar1=1.0,
                                scalar2=None, op0=Alu.min)

        # ---------- one-time weight prep ----------
        with tc.tile_pool(name="wprep", bufs=1) as wp:
            Wq_s = wp.tile([64, H], f32)
            nc.scalar.dma_start(out=Wq_s[:], in_=Wq_d.ap())
            Wkx_s = wp.tile([64, H + 1], f32)
            nc.scalar.dma_start(out=Wkx_s[:, 0:H], in_=Wk_d.ap())
            nc.scalar.dma_start(out=Wkx_s[:, H:H + 1], in_=bk_d.ap())
            bq_s = wp.tile([64, 1], f32)
            nc.scalar.dma_start(out=bq_s[:], in_=bq_d.ap())
            Wo_s = wp.tile([3, 131], f32)
            nc.scalar.dma_start(out=Wo_s[:], in_=Wo_d.ap())
            Wc0_s = wp.tile([P, 131], f32)
            nc.scalar.dma_start(out=Wc0_s[:], in_=Wc_d.ap()[0:P, :])
            Wc1_s = wp.tile([3, 131], f32)
            nc.scalar.dma_start(out=Wc1_s[:], in_=Wc_d.ap()[P:131, :])
            bc0_s = wp.tile([P, 1], f32)
            nc.scalar.dma_start(out=bc0_s[:], in_=bc_d.ap()[0:P, :])
            bc1_s = wp.tile([3, 1], f32)
            nc.scalar.dma_start(out=bc1_s[:], in_=bc_d.ap()[P:131, :])
            bo_s = wp.tile([1, 3], f32)
            nc.scalar.dma_start(out=bo_s[:], in_=bo_d.ap())

            with tc.tile_pool(name="wprep_psA", bufs=1, space="PSUM") as wpp:
                woT0_p = wpp.tile([P, 3], f32, space="PSUM")
                nc.tensor.transpose(out=woT0_p[:], in_=Wo_s[:, 0:P],
                                    identity=idn[0:3, 0:3])
                woT0_s = wp.tile([P, 3], f32)
                nc.vector.tensor_copy(out=woT0_s[:], in_=woT0_p[:])
                woT1_p = wpp.tile([3, 3], f32, space="PSUM")
                nc.tensor.transpose(out=woT1_p[:], in_=Wo_s[:, P:131],
                                    identity=idn[0:3, 0:3])
                woT1_s = wp.tile([3, 3], f32)
                nc.vector.tensor_copy(out=woT1_s[:], in_=woT1_p[:])

                # Wf = Wo @ Wc  [3,131]
                Wf_p = wpp.tile([3, 131], f32, space="PSUM")
                nc.tensor.matmul(out=Wf_p[:], lhsT=woT0_s[:], rhs=Wc0_s[:],
                                 start=True, stop=False)
                nc.tensor.matmul(out=Wf_p[:], lhsT=woT1_s[:], rhs=Wc1_s[:],
                                 start=False, stop=True)
                Wf_s = wp.tile([3, 131], f32)
                nc.vector.tensor_copy(out=Wf_s[:], in_=Wf_p[:])

                # bfT = (Wo @ bc).T [1,3] ; + bo
                bfT_p = wpp.tile([1, 3], f32, space="PSUM")
                nc.tensor.matmul(out=bfT_p[:], lhsT=bc0_s[:], rhs=woT0_s[:],
                                 start=True, stop=False)
                nc.tensor.matmul(out=bfT_p[:], lhsT=bc1_s[:], rhs=woT1_s[:],
                                 start=False, stop=True)
                bfT_s = wp.tile([1, 3], f32)
                nc.vector.tensor_add(out=bfT_s[:], in0=bfT_p[:], in1=bo_s[:])

            with tc.tile_pool(name="wprep_psB", bufs=1, space="PSUM") as wpp2:
                # wfx rows: [Wf[c,0:131] | 0 | I3[c,:]-Wf[c,128:131] | bf[c]],
                # replicated across partitions (stride-0 partition APs are
                # illegal on DVE): e_c row extract + ones-outer-product.
                # The bias column pairs with caggx[:,135] = sum(exp), so the
                # whole output is one matvec scaled by 1/sum(exp) at the end.
                wfx_s = const.tile([P, 3, 136], f32)
                nc.vector.memset(wfx_s[:], 0.0)
                for c in range(3):
                    row_p = wpp2.tile([1, 131], f32, space="PSUM",
                                      name="row_p")
                    nc.tensor.matmul(out=row_p[:], lhsT=idn[0:3, c:c + 1],
                                     rhs=Wf_s[:], start=True, stop=True)
                    row_s = wp.tile([1, 136], f32, name=f"row_s{c}")
                    nc.vector.memset(row_s[:], 0.0)
                    nc.scalar.activation(out=row_s[:, 0:131], in_=row_p[:],
                                         func=Act.Copy)
                    # cols 132..134: I3[c,:] - Wf[c,128:131]
                    rowI = wp.tile([1, 3], f32, name=f"rowI{c}")
                    nc.vector.memset(rowI[:], 0.0)
                    nc.vector.memset(rowI[:, c:c + 1], 1.0)
                    nc.vector.tensor_sub(out=row_s[:, 132:135],
                                         in0=rowI[:],
                                         in1=row_s[:, 128:131])
                    # col 135: bf[c]
                    nc.scalar.activation(out=row_s[:, 135:136],
                                         in_=bfT_s[:, c:c + 1],
                                         func=Act.Copy)
                    wfx_p = wpp2.tile([P, 136], f32, space="PSUM",
                                      name="wfx_p")
                    nc.tensor.matmul(out=wfx_p[:], lhsT=ones1[:],
                                     rhs=row_s[:], start=True, stop=True)
                    nc.scalar.activation(out=wfx_s[:, c, :], in_=wfx_p[:],
                                         func=Act.Copy)

                # M_ext = [Wq.T @ Wk | Wq.T @ bk] / scale  [128, 129]
                Mw_p = wpp2.tile([P, H + 1], f32, space="PSUM")
                nc.tensor.matmul(out=Mw_p[:], lhsT=Wq_s[:], rhs=Wkx_s[:],
                                 start=True, stop=True)
                Mx_s = const.tile([P, H + 1], f32)
                nc.scalar.activation(out=Mx_s[:], in_=Mw_p[:], func=Act.Copy,
                                     scale=1.0 / SCALE)

                # [c1 | s2] = [bq @ Wk | bq.bk] / scale  [1, 129]
                cs_p = wpp2.tile([1, H + 1], f32, space="PSUM")
                nc.tensor.matmul(out=cs_p[:], lhsT=bq_s[:], rhs=Wkx_s[:],
                                 start=True, stop=True)
                cs_s = const.tile([1, H + 1], f32)
                nc.scalar.activation(out=cs_s[:], in_=cs_p[:], func=Act.Copy,
                                     scale=1.0 / SCALE)

        # ---------- main loop ----------
        sb = ctx.enter_context(tc.tile_pool(name="sb", bufs=4))
        gp = ctx.enter_context(tc.tile_pool(name="gp", bufs=8))
        big = ctx.enter_context(tc.tile_pool(name="big", bufs=4))
        dg = ctx.enter_context(tc.tile_pool(name="dg", bufs=4))
        pp = ctx.enter_context(tc.tile_pool(name="pp", bufs=4, space="PSUM"))

        for t in range(NT):
            rows = slice(t * P, (t + 1) * P)

            xT_t = sb.tile([P, P], f32)
            nc.sync.dma_start(out=xT_t[:], in_=xT_d.ap()[:, rows])
            idx_t = idx_all[:, t * K:(t + 1) * K]
            pts_t = sb.tile([P, 4], f32)
            nc.sync.dma_start(out=pts_t[:], in_=pts_d.ap()[rows, :])

            # K single-index-per-partition indirect gathers (128 rows each),
            # spread over the 4 SWDGE queues for descriptor-ring headroom.
            g_t = gp.tile([P, K, TW], bf16)
            for k in range(K):
                nc.gpsimd.indirect_dma_start(
                    out=g_t[:, k, :],
                    out_offset=None,
                    in_=tab_d.ap(),
                    in_offset=bass.IndirectOffsetOnAxis(
                        ap=idx_t[:, k:k + 1], axis=0),
                )

            # u_ext = xT.T @ M_ext + bcast([c1|s2])  ->  [p, 129] = [u | s]
            # (kept in PSUM: DVE reads of it avoid the shared SBUF port)
            u_p = pp.tile([P, H + 1], f32, space="PSUM")
            nc.tensor.matmul(out=u_p[:], lhsT=xT_t[:], rhs=Mx_s[:],
                             start=True, stop=False)
            nc.tensor.matmul(out=u_p[:], lhsT=ones1[:], rhs=cs_s[:],
                             start=False, stop=True)
            s_f = sb.tile([P, 1], f32)
            nc.scalar.activation(out=s_f[:], in_=u_p[:, H:H + 1],
                                 func=Act.Copy)

            # scores: raw[p,k] = sum_h u[p,h] * g[p,k,h]  (+ s, * mask)
            # (split so only the last 3 slots' product trails the final
            # gather — shortens the per-tile critical chain)
            prod = big.tile([P, K, H], bf16)
            nc.vector.tensor_mul(
                out=prod[:, 0:12, :],
                in0=g_t[:][:, 0:12, 0:H],
                in1=u_p[:, 0:H].unsqueeze(1).to_broadcast([P, 12, H]),
            )
            nc.vector.tensor_mul(
                out=prod[:, 12:K, :],
                in0=g_t[:][:, 12:K, 0:H],
                in1=u_p[:, 0:H].unsqueeze(1).to_broadcast([P, K - 12, H]),
            )
            raw = sb.tile([P, K], f32)
            nc.vector.tensor_reduce(out=raw[:, 0:12], in_=prod[:, 0:12, :],
                                    axis=mybir.AxisListType.X, op=Alu.add)
            nc.vector.tensor_reduce(out=raw[:, 12:K], in_=prod[:, 12:K, :],
                                    axis=mybir.AxisListType.X, op=Alu.add)
            sc = sb.tile([P, K], f32)
            nc.vector.tensor_scalar(out=sc[:], in0=raw[:],
                                    scalar1=s_f[:], scalar2=None,
                                    op0=Alu.add)
            scm = sb.tile([P, K], f32)
            nc.vector.tensor_mul(out=scm[:], in0=sc[:],
                                 in1=mask_all[:, t * K:(t + 1) * K])

            # softmax over k (scores are bounded |.|<~8, skip max-sub).
            # The normalization 1/sum(exp) is applied once at the very end,
            # so the aggregation uses raw exp weights and the reciprocal
            # stays off the critical chain.
            e_t = sb.tile([P, K], f32)
            se_t = sb.tile([P, 1], f32)
            nc.scalar.activation(out=e_t[:], in_=scm[:], func=Act.Exp,
                                 bias=0.0, scale=1.0, accum_out=se_t[:])
            r_t = sb.tile([P, 1], f32)
            nc.vector.reciprocal(out=r_t[:], in_=se_t[:])

            # caggx = [sum_k e_k * g_k (132) | pts*S (3) | S],  S = sum(exp),
            # accumulated on PE: caggx += diag(e_k) @ g_k.  diag built on
            # the Scalar engine (per-partition scale of identity).
            caggx_p = pp.tile([P, 136], f32, space="PSUM")
            for k in range(K):
                dg_t = dg.tile([P, P], bf16)
                nc.scalar.activation(out=dg_t[:], in_=idn[:], func=Act.Copy,
                                     scale=e_t[:, k:k + 1])
                nc.tensor.matmul(out=caggx_p[:, 0:132], lhsT=dg_t[:],
                                 rhs=g_t[:, k, :], start=(k == 0),
                                 stop=(k == K - 1))
            nc.scalar.activation(out=caggx_p[:, 132:135], in_=pts_t[:, 0:3],
                                 func=Act.Copy, scale=se_t[:])
            nc.scalar.activation(out=caggx_p[:, 135:136], in_=se_t[:],
                                 func=Act.Copy)

            # disp[p,c] = Wfx[c,:] . caggx[p,:]  (PSUM-side mul avoids the
            # DVE<->GpSimd shared SBUF port); out = disp / S
            disp = sb.tile([P, 3], f32)
            junk = sb.tile([P, 136], f32)
            for c in range(3):
                nc.vector.tensor_mul(out=junk[:], in0=caggx_p[:, 0:136],
                                     in1=wfx_s[:, c, :])
                nc.vector.tensor_reduce(out=disp[:, c:c + 1], in_=junk[:],
                                        axis=mybir.AxisListType.X,
                                        op=Alu.add)

            out_t = sb.tile([P, 4], f32)
            nc.scalar.activation(out=out_t[:, 0:3], in_=disp[:],
                                 func=Act.Copy, scale=r_t[:])
            nc.sync.dma_start(out=out_d.ap()[rows, 0:3], in_=out_t[:, 0:3])

    nc.compile()
    return nc


def get_nc():
    if "nc" not in _NC_CACHE:
        _NC_CACHE["nc"] = build_nc()
    return _NC_CACHE["nc"]


def make_in_maps(sampled_points, sampled_x, Wq, bq, Wk, bk, Wc, bc, Wo, bo,
                 edge_index_filtered):
    sampled_points = np.ascontiguousarray(sampled_points, np.float32)
    sampled_x = np.ascontiguousarray(sampled_x, np.float32)
    nbr = np.ascontiguousarray(
        np.asarray(edge_index_filtered)[1].reshape(N, K)).astype(np.int32)

    import ml_dtypes
    tab = np.zeros((N, TW), ml_dtypes.bfloat16)
    tab[:, :H] = sampled_x.astype(ml_dtypes.bfloat16)
    tab[:, H:H + 3] = sampled_points.astype(ml_dtypes.bfloat16)

    shared = {
        "tab": tab,
        "idn": np.eye(P, dtype=np.float32),
        "Wq": np.ascontiguousarray(Wq, np.float32),
        "Wk": np.ascontiguousarray(Wk, np.float32),
        "bq": np.ascontiguousarray(np.reshape(bq, (64, 1)), np.float32),
        "bk": np.ascontiguousarray(np.reshape(bk, (64, 1)), np.float32),
        "Wc": np.ascontiguousarray(Wc, np.float32),
        "bc": np.ascontiguousarray(np.reshape(bc, (131, 1)), np.float32),
        "Wo": np.ascontiguousarray(Wo, np.float32),
        "bo": np.ascontiguousarray(np.reshape(bo, (1, 3)), np.float32),
    }

    in_maps = []
    for c in range(NCORES):
        rows = slice(c * SH, (c + 1) * SH)
        xT = np.zeros((P, NP), np.float32)
        xT[:, :SH] = sampled_x[rows].T
        pts4 = np.zeros((NP, 4), np.float32)
        pts4[:SH, :3] = sampled_points[rows]
        nb = np.zeros((NP, K), np.int32)
        nb[:SH] = nbr[rows]
        # [P, NT*K]: column t*K+k = nbr[t*128+p, k]
        idx = np.ascontiguousarray(
            nb.reshape(NT, P, K).transpose(1, 0, 2).reshape(P, NT * K))
        in_maps.append({**shared, "xT": xT, "pts": pts4, "idx": idx})
    return in_maps


def unshard(results):
    out = np.concatenate(
        [results[c]["out"][:SH, :3] for c in range(NCORES)], axis=0)
    return np.ascontiguousarray(out)


def kernel(**inputs):
    from concourse.bass_utils import run_bass_kernel_spmd

    nc = get_nc()
    in_maps = make_in_maps(**inputs)
    res = run_bass_kernel_spmd(nc, in_maps, core_ids=list(range(NCORES)))
    return unshard(res.results)


# revision 15
# speedup vs baseline: 1.1786x; 1.0012x over previous
"""GNN attention layer (N=50000 nodes, K=15 neighbors, H=128) on 8 TRN2 cores.

Math (reference):
    nbr = dst.reshape(N, K)
    q  = x @ Wq.T + bq                      # [N, 64]
    kf = x[nbr] @ Wk.T + bk                 # [N, K, 64]
    scores = (q . kf) / scale               # [N, K]
    attn = softmax(scores * (nbr != 0))     # [N, K]
    cagg = sum_k attn * [x[nbr], pts[nbr] - pts]   # [N, 131]
    out  = pts + (cagg @ Wc.T + bc) @ Wo.T + bo

Algebraic restructuring (exact up to fp assoc):
  * scores[i,k] = u[i] . x[nbr[i,k]] + s[i], with
        u = (x @ (Wq.T @ Wk) + bq @ Wk) / scale      # [N, 128]
        s = (x @ (Wq.T @ bk) + bq.bk) / scale        # [N]
    (s must be added before the mask multiply).
  * since sum_k attn = 1:
        disp = Wf @ cagg + bf,  Wf = Wo @ Wc [3,131], bf = Wo @ bc + bo
        cagg_p = (sum_k attn * pts[nbr]) - pts
    The -pts and +pts(residual) terms are folded into an extended matvec:
        out[p,c] = sum_f Wfx[c,f]*caggx[p,f] + bf[c]
    where caggx = [sum_k attn*tabrow(132) | pts(3)] (PSUM, PE-accumulated)
    and Wfx[c,:] = [Wf[c,0:131] | 0 | I3[c,:] - Wf[c,128:131]].

Gather: this image has no GPSIMD extended-instruction ucode (bedrock), and
the indirect1d ucode only supports ONE int32 index per partition (multi-
index offset APs degenerate to a single lane; HW-probed), so each gather
instruction fetches 128 rows from a fused bf16 table [x(128)|pts(3)|pad]
(528B... 264B rows).  A 128-node tile needs K=15 gathers.  Descriptor
generation serializes on the Pool engine (~1.1us/instr) — that is the hard
floor.  Everything else is kept OFF the DVE<->GpSimd shared SBUF port pair
(an exclusive, full-instruction lock that stalls SWDGE desc-gen):
  * DVE two-tensor ops read one operand from PSUM (u, caggx) — no shared
    port use.
  * The attention-weighted aggregation runs on PE as 15 PSUM-accumulated
    diag(attn_k) @ g_k matmuls; diag matrices are built by the Scalar
    engine (per-partition scale of a cached identity).
  * softmax scaling and small copies run on the Scalar engine.

Sharding: nodes split contiguously over 8 cores (6250 each, padded to
6272 = 49*128); the gather table is replicated per core. No collectives.
"""

import numpy as np

N = 50000
K = 15
H = 128
NCORES = 8
SH = N // NCORES          # 6250 real nodes per core
P = 128
NT = 49                   # tiles per core
NP = NT * P               # 6272 padded nodes per core
TW = H + 4                # table row width [x 128 | pts 3 | pad]
SCALE = float(np.sqrt(64.0) + 1e-6)

_NC_CACHE = {}


def build_nc():
    import contextlib

    import concourse.bacc as bacc
    import concourse.bass as bass
    import concourse.mybir as mybir
    import concourse.tile as tile

    f32 = mybir.dt.float32
    bf16 = mybir.dt.bfloat16
    i32 = mybir.dt.int32
    Alu = mybir.AluOpType
    Act = mybir.ActivationFunctionType

    nc = bacc.Bacc("TRN2", target_bir_lowering=False, debug=False,
                   num_devices=NCORES, dynamic_dma_scratch_size=65536,
                   num_swdge_queues=1)

    tab_d = nc.dram_tensor("tab", [N, TW], bf16, kind="ExternalInput")
    xT_d = nc.dram_tensor("xT", [P, NP], f32, kind="ExternalInput")
    pts_d = nc.dram_tensor("pts", [NP, 4], f32, kind="ExternalInput")
    idx_d = nc.dram_tensor("idx", [P, NT * K], i32, kind="ExternalInput")
    Wq_d = nc.dram_tensor("Wq", [64, H], f32, kind="ExternalInput")
    Wk_d = nc.dram_tensor("Wk", [64, H], f32, kind="ExternalInput")
    bq_d = nc.dram_tensor("bq", [64, 1], f32, kind="ExternalInput")
    bk_d = nc.dram_tensor("bk", [64, 1], f32, kind="ExternalInput")
    Wc_d = nc.dram_tensor("Wc", [131, 131], f32, kind="ExternalInput")
    bc_d = nc.dram_tensor("bc", [131, 1], f32, kind="ExternalInput")
    Wo_d = nc.dram_tensor("Wo", [3, 131], f32, kind="ExternalInput")
    bo_d = nc.dram_tensor("bo", [1, 3], f32, kind="ExternalInput")
    idn_d = nc.dram_tensor("idn", [P, P], f32, kind="ExternalInput")
    out_d = nc.dram_tensor("out", [NP, 4], f32, kind="ExternalOutput")

    with tile.TileContext(nc) as tc, contextlib.ExitStack() as ctx:
        const = ctx.enter_context(tc.tile_pool(name="const", bufs=1))

        ones1 = const.tile([1, P], f32)
        nc.vector.memset(ones1[:], 1.0)

        # identity shipped as an input (keeps make_identity's gpsimd ops off
        # the Pool engine ahead of the first gather)
        idn = const.tile([P, P], f32)
        nc.scalar.dma_start(out=idn[:], in_=idn_d.ap())

        # all gather indices, preloaded once: column t*K+k holds nbr[t*128+p, k]
        idx_all = const.tile([P, NT * K], i32)
        nc.sync.dma_start(out=idx_all[:], in_=idx_d.ap())
        # mask = (nbr != 0) == min(idx, 1) for idx >= 0, built once for all
        # tiles (keeps per-tile DVE work off the shared SBUF port).
        idxf_all = const.tile([P, NT * K], f32)
        nc.vector.tensor_copy(out=idxf_all[:], in_=idx_all[:])
        mask_all = const.tile([P, NT * K], f32)
        nc.vector.tensor_scalar(out=mask_all[:], in0=idxf_all[:], scalar1=1.0,
                                scalar2=None, op0=Alu.min)

        # ---------- one-time weight prep ----------
        with tc.tile_pool(name="wprep", bufs=1) as wp:
            Wq_s = wp.tile([64, H], f32)
            nc.scalar.dma_start(out=Wq_s[:], in_=Wq_d.ap())
            Wkx_s = wp.tile([64, H + 1], f32)
            nc.scalar.dma_start(out=Wkx_s[:, 0:H], in_=Wk_d.ap())
            nc.scalar.dma_start(out=Wkx_s[:, H:H + 1], in_=bk_d.ap())
            bq_s = wp.tile([64, 1], f32)
            nc.scalar.dma_start(out=bq_s[:], in_=bq_d.ap())
            Wo_s = wp.tile([3, 131], f32)
            nc.scalar.dma_start(out=Wo_s[:], in_=Wo_d.ap())
            Wc0_s = wp.tile([P, 131], f32)
            nc.scalar.dma_start(out=Wc0_s[:], in_=Wc_d.ap()[0:P, :])
            Wc1_s = wp.tile([3, 131], f32)
            nc.scalar.dma_start(out=Wc1_s[:], in_=Wc_d.ap()[P:131, :])
            bc0_s = wp.tile([P, 1], f32)
            nc.scalar.dma_start(out=bc0_s[:], in_=bc_d.ap()[0:P, :])
            bc1_s = wp.tile([3, 1], f32)
            nc.scalar.dma_start(out=bc1_s[:], in_=bc_d.ap()[P:131, :])
            bo_s = wp.tile([1, 3], f32)
            nc.scalar.dma_start(out=bo_s[:], in_=bo_d.ap())

            with tc.tile_pool(name="wprep_psA", bufs=1, space="PSUM") as wpp:
                woT0_p = wpp.tile([P, 3], f32, space="PSUM")
                nc.tensor.transpose(out=woT0_p[:], in_=Wo_s[:, 0:P],
                                    identity=idn[0:3, 0:3])
                woT0_s = wp.tile([P, 3], f32)
                nc.vector.tensor_copy(out=woT0_s[:], in_=woT0_p[:])
                woT1_p = wpp.tile([3, 3], f32, space="PSUM")
                nc.tensor.transpose(out=woT1_p[:], in_=Wo_s[:, P:131],
                                    identity=idn[0:3, 0:3])
                woT1_s = wp.tile([3, 3], f32)
                nc.vector.tensor_copy(out=woT1_s[:], in_=woT1_p[:])

                # Wf = Wo @ Wc  [3,131]
                Wf_p = wpp.tile([3, 131], f32, space="PSUM")
                nc.tensor.matmul(out=Wf_p[:], lhsT=woT0_s[:], rhs=Wc0_s[:],
                                 start=True, stop=False)
                nc.tensor.matmul(out=Wf_p[:], lhsT=woT1_s[:], rhs=Wc1_s[:],
                                 start=False, stop=True)
                Wf_s = wp.tile([3, 131], f32)
                nc.vector.tensor_copy(out=Wf_s[:], in_=Wf_p[:])

                # bfT = (Wo @ bc).T [1,3] ; + bo
                bfT_p = wpp.tile([1, 3], f32, space="PSUM")
                nc.tensor.matmul(out=bfT_p[:], lhsT=bc0_s[:], rhs=woT0_s[:],
                                 start=True, stop=False)
                nc.tensor.matmul(out=bfT_p[:], lhsT=bc1_s[:], rhs=woT1_s[:],
                                 start=False, stop=True)
                bfT_s = wp.tile([1, 3], f32)
                nc.vector.tensor_add(out=bfT_s[:], in0=bfT_p[:], in1=bo_s[:])

            with tc.tile_pool(name="wprep_psB", bufs=1, space="PSUM") as wpp2:
                # wfx rows: [Wf[c,0:131] | 0 | I3[c,:]-Wf[c,128:131] | bf[c]],
                # replicated across partitions (stride-0 partition APs are
                # illegal on DVE): e_c row extract + ones-outer-product.
                # The bias column pairs with caggx[:,135] = sum(exp), so the
                # whole output is one matvec scaled by 1/sum(exp) at the end.
                wfx_s = const.tile([P, 3, 136], f32)
                nc.vector.memset(wfx_s[:], 0.0)
                for c in range(3):
                    row_p = wpp2.tile([1, 131], f32, space="PSUM",
                                      name="row_p")
                    nc.tensor.matmul(out=row_p[:], lhsT=idn[0:3, c:c + 1],
                                     rhs=Wf_s[:], start=True, stop=True)
                    row_s = wp.tile([1, 136], f32, name=f"row_s{c}")
                    nc.vector.memset(row_s[:], 0.0)
                    nc.scalar.activation(out=row_s[:, 0:131], in_=row_p[:],
                                         func=Act.Copy)
                    # cols 132..134: I3[c,:] - Wf[c,128:131]
                    rowI = wp.tile([1, 3], f32, name=f"rowI{c}")
                    nc.vector.memset(rowI[:], 0.0)
                    nc.vector.memset(rowI[:, c:c + 1], 1.0)
                    nc.vector.tensor_sub(out=row_s[:, 132:135],
                                         in0=rowI[:],
                                         in1=row_s[:, 128:131])
                    # col 135: bf[c]
                    nc.scalar.activation(out=row_s[:, 135:136],
                                         in_=bfT_s[:, c:c + 1],
                                         func=Act.Copy)
                    wfx_p = wpp2.tile([P, 136], f32, space="PSUM",
                                      name="wfx_p")
                    nc.tensor.matmul(out=wfx_p[:], lhsT=ones1[:],
                                     rhs=row_s[:], start=True, stop=True)
                    nc.scalar.activation(out=wfx_s[:, c, :], in_=wfx_p[:],
                                         func=Act.Copy)

                # M_ext = [Wq.T @ Wk | Wq.T @ bk] / scale  [128, 129]
                Mw_p = wpp2.tile([P, H + 1], f32, space="PSUM")
                nc.tensor.matmul(out=Mw_p[:], lhsT=Wq_s[:], rhs=Wkx_s[:],
                                 start=True, stop=True)
                Mx_s = const.tile([P, H + 1], f32)
                nc.scalar.activation(out=Mx_s[:], in_=Mw_p[:], func=Act.Copy,
                                     scale=1.0 / SCALE)

                # [c1 | s2] = [bq @ Wk | bq.bk] / scale  [1, 129]
                cs_p = wpp2.tile([1, H + 1], f32, space="PSUM")
                nc.tensor.matmul(out=cs_p[:], lhsT=bq_s[:], rhs=Wkx_s[:],
                                 start=True, stop=True)
                cs_s = const.tile([1, H + 1], f32)
                nc.scalar.activation(out=cs_s[:], in_=cs_p[:], func=Act.Copy,
                                     scale=1.0 / SCALE)

        # ---------- main loop ----------
        sb = ctx.enter_context(tc.tile_pool(name="sb", bufs=4))
        gp = ctx.enter_context(tc.tile_pool(name="gp", bufs=8))
        big = ctx.enter_context(tc.tile_pool(name="big", bufs=4))
        dg = ctx.enter_context(tc.tile_pool(name="dg", bufs=4))
        pp = ctx.enter_context(tc.tile_pool(name="pp", bufs=4, space="PSUM"))

        for t in range(NT):
            rows = slice(t * P, (t + 1) * P)

            xT_t = sb.tile([P, P], f32)
            nc.sync.dma_start(out=xT_t[:], in_=xT_d.ap()[:, rows])
            idx_t = idx_all[:, t * K:(t + 1) * K]
            pts_t = sb.tile([P, 4], f32)
            nc.sync.dma_start(out=pts_t[:], in_=pts_d.ap()[rows, :])

            # K single-index-per-partition indirect gathers (128 rows each),
            # spread over the 4 SWDGE queues for descriptor-ring headroom.
            g_t = gp.tile([P, K, TW], bf16)
            for k in range(K):
                nc.gpsimd.indirect_dma_start(
                    out=g_t[:, k, :],
                    out_offset=None,
                    in_=tab_d.ap(),
                    in_offset=bass.IndirectOffsetOnAxis(
                        ap=idx_t[:, k:k + 1], axis=0),
                )

            # u_ext = xT.T @ M_ext + bcast([c1|s2])  ->  [p, 129] = [u | s]
            # (kept in PSUM: DVE reads of it avoid the shared SBUF port)
            u_p = pp.tile([P, H + 1], f32, space="PSUM")
            nc.tensor.matmul(out=u_p[:], lhsT=xT_t[:], rhs=Mx_s[:],
                             start=True, stop=False)
            nc.tensor.matmul(out=u_p[:], lhsT=ones1[:], rhs=cs_s[:],
                             start=False, stop=True)
            s_f = sb.tile([P, 1], f32)
            nc.scalar.activation(out=s_f[:], in_=u_p[:, H:H + 1],
                                 func=Act.Copy)

            # scores: raw[p,k] = sum_h u[p,h] * g[p,k,h]  (+ s, * mask)
            # (split so only the last 3 slots' product trails the final
            # gather — shortens the per-tile critical chain)
            prod = big.tile([P, K, H], bf16)
            nc.vector.tensor_mul(
                out=prod[:, 0:12, :],
                in0=g_t[:][:, 0:12, 0:H],
                in1=u_p[:, 0:H].unsqueeze(1).to_broadcast([P, 12, H]),
            )
            nc.vector.tensor_mul(
                out=prod[:, 12:K, :],
                in0=g_t[:][:, 12:K, 0:H],
                in1=u_p[:, 0:H].unsqueeze(1).to_broadcast([P, K - 12, H]),
            )
            raw = sb.tile([P, K], f32)
            nc.vector.tensor_reduce(out=raw[:, 0:12], in_=prod[:, 0:12, :],
                                    axis=mybir.AxisListType.X, op=Alu.add)
            nc.vector.tensor_reduce(out=raw[:, 12:K], in_=prod[:, 12:K, :],
                                    axis=mybir.AxisListType.X, op=Alu.add)
            # scores + softmax numerators, split 12+3 like the product so
            # the first 12 diag/matmul aggregation steps run before the
            # tile's last gather lands.  The normalization 1/sum(exp) is
            # applied once at the very end (aggregation uses raw exp
            # weights), with sum(exp) = se_a + se_b.
            sc = sb.tile([P, K], f32)
            scm = sb.tile([P, K], f32)
            e_t = sb.tile([P, K], f32)
            se_a = sb.tile([P, 1], f32)
            se_b = sb.tile([P, 1], f32)
            nc.vector.tensor_scalar(out=sc[:, 0:12], in0=raw[:, 0:12],
                                    scalar1=s_f[:], scalar2=None,
                                    op0=Alu.add)
            nc.vector.tensor_mul(out=scm[:, 0:12], in0=sc[:, 0:12],
                                 in1=mask_all[:, t * K:t * K + 12])
            nc.scalar.activation(out=e_t[:, 0:12], in_=scm[:, 0:12],
                                 func=Act.Exp, bias=0.0, scale=1.0,
                                 accum_out=se_a[:])
            nc.vector.tensor_scalar(out=sc[:, 12:K], in0=raw[:, 12:K],
                                    scalar1=s_f[:], scalar2=None,
                                    op0=Alu.add)
            nc.vector.tensor_mul(out=scm[:, 12:K], in0=sc[:, 12:K],
                                 in1=mask_all[:, t * K + 12:(t + 1) * K])
            nc.scalar.activation(out=e_t[:, 12:K], in_=scm[:, 12:K],
                                 func=Act.Exp, bias=0.0, scale=1.0,
                                 accum_out=se_b[:])
            se_t = sb.tile([P, 1], f32)
            nc.vector.tensor_add(out=se_t[:], in0=se_a[:], in1=se_b[:])
            r_t = sb.tile([P, 1], f32)
            nc.vector.reciprocal(out=r_t[:], in_=se_t[:])

            # caggx = [sum_k e_k * g_k (132) | pts*S (3) | S],  S = sum(exp),
            # accumulated on PE: caggx += diag(e_k) @ g_k.  diag built on
            # the Scalar engine (per-partition scale of identity).
            caggx_p = pp.tile([P, 136], f32, space="PSUM")
            for k in range(K):
                dg_t = dg.tile([P, P], bf16)
                nc.scalar.activation(out=dg_t[:], in_=idn[:], func=Act.Copy,
                                     scale=e_t[:, k:k + 1])
                nc.tensor.matmul(out=caggx_p[:, 0:132], lhsT=dg_t[:],
                                 rhs=g_t[:, k, :], start=(k == 0),
                                 stop=(k == K - 1))
            nc.scalar.activation(out=caggx_p[:, 132:135], in_=pts_t[:, 0:3],
                                 func=Act.Copy, scale=se_t[:])
            nc.scalar.activation(out=caggx_p[:, 135:136], in_=se_t[:],
                                 func=Act.Copy)

            # disp[p,c] = Wfx[c,:] . caggx[p,:]  (PSUM-side mul avoids the
            # DVE<->GpSimd shared SBUF port); out = disp / S
            disp = sb.tile([P, 3], f32)
            junk = sb.tile([P, 136], f32)
            for c in range(3):
                nc.vector.tensor_mul(out=junk[:], in0=caggx_p[:, 0:136],
                                     in1=wfx_s[:, c, :])
                nc.vector.tensor_reduce(out=disp[:, c:c + 1], in_=junk[:],
                                        axis=mybir.AxisListType.X,
                                        op=Alu.add)

            out_t = sb.tile([P, 4], f32)
            nc.scalar.activation(out=out_t[:, 0:3], in_=disp[:],
                                 func=Act.Copy, scale=r_t[:])
            nc.sync.dma_start(out=out_d.ap()[rows, 0:3], in_=out_t[:, 0:3])

    nc.compile()
    return nc


def get_nc():
    if "nc" not in _NC_CACHE:
        _NC_CACHE["nc"] = build_nc()
    return _NC_CACHE["nc"]


def make_in_maps(sampled_points, sampled_x, Wq, bq, Wk, bk, Wc, bc, Wo, bo,
                 edge_index_filtered):
    sampled_points = np.ascontiguousarray(sampled_points, np.float32)
    sampled_x = np.ascontiguousarray(sampled_x, np.float32)
    nbr = np.ascontiguousarray(
        np.asarray(edge_index_filtered)[1].reshape(N, K)).astype(np.int32)

    import ml_dtypes
    tab = np.zeros((N, TW), ml_dtypes.bfloat16)
    tab[:, :H] = sampled_x.astype(ml_dtypes.bfloat16)
    tab[:, H:H + 3] = sampled_points.astype(ml_dtypes.bfloat16)

    shared = {
        "tab": tab,
        "idn": np.eye(P, dtype=np.float32),
        "Wq": np.ascontiguousarray(Wq, np.float32),
        "Wk": np.ascontiguousarray(Wk, np.float32),
        "bq": np.ascontiguousarray(np.reshape(bq, (64, 1)), np.float32),
        "bk": np.ascontiguousarray(np.reshape(bk, (64, 1)), np.float32),
        "Wc": np.ascontiguousarray(Wc, np.float32),
        "bc": np.ascontiguousarray(np.reshape(bc, (131, 1)), np.float32),
        "Wo": np.ascontiguousarray(Wo, np.float32),
        "bo": np.ascontiguousarray(np.reshape(bo, (1, 3)), np.float32),
    }

    in_maps = []
    for c in range(NCORES):
        rows = slice(c * SH, (c + 1) * SH)
        xT = np.zeros((P, NP), np.float32)
        xT[:, :SH] = sampled_x[rows].T
        pts4 = np.zeros((NP, 4), np.float32)
        pts4[:SH, :3] = sampled_points[rows]
        nb = np.zeros((NP, K), np.int32)
        nb[:SH] = nbr[rows]
        # [P, NT*K]: column t*K+k = nbr[t*128+p, k]
        idx = np.ascontiguousarray(
            nb.reshape(NT, P, K).transpose(1, 0, 2).reshape(P, NT * K))
        in_maps.append({**shared, "xT": xT, "pts": pts4, "idx": idx})
    return in_maps


def unshard(results):
    out = np.concatenate(
        [results[c]["out"][:SH, :3] for c in range(NCORES)], axis=0)
    return np.ascontiguousarray(out)


def kernel(**inputs):
    from concourse.bass_utils import run_bass_kernel_spmd

    nc = get_nc()
    in_maps = make_in_maps(**inputs)
    res = run_bass_kernel_spmd(nc, in_maps, core_ids=list(range(NCORES)))
    return unshard(res.results)


# revision 16
# speedup vs baseline: 1.1878x; 1.0079x over previous
"""GNN attention layer (N=50000 nodes, K=15 neighbors, H=128) on 8 TRN2 cores.

Math (reference):
    nbr = dst.reshape(N, K)
    q  = x @ Wq.T + bq                      # [N, 64]
    kf = x[nbr] @ Wk.T + bk                 # [N, K, 64]
    scores = (q . kf) / scale               # [N, K]
    attn = softmax(scores * (nbr != 0))     # [N, K]
    cagg = sum_k attn * [x[nbr], pts[nbr] - pts]   # [N, 131]
    out  = pts + (cagg @ Wc.T + bc) @ Wo.T + bo

Algebraic restructuring (exact up to fp assoc):
  * scores[i,k] = u[i] . x[nbr[i,k]] + s[i], with
        u = (x @ (Wq.T @ Wk) + bq @ Wk) / scale      # [N, 128]
        s = (x @ (Wq.T @ bk) + bq.bk) / scale        # [N]
    (s must be added before the mask multiply).
  * since sum_k attn = 1:
        disp = Wf @ cagg + bf,  Wf = Wo @ Wc [3,131], bf = Wo @ bc + bo
        cagg_p = (sum_k attn * pts[nbr]) - pts
    The -pts and +pts(residual) terms are folded into an extended matvec:
        out[p,c] = sum_f Wfx[c,f]*caggx[p,f] + bf[c]
    where caggx = [sum_k attn*tabrow(132) | pts(3)] (PSUM, PE-accumulated)
    and Wfx[c,:] = [Wf[c,0:131] | 0 | I3[c,:] - Wf[c,128:131]].

Gather: this image has no GPSIMD extended-instruction ucode (bedrock), and
the indirect1d ucode only supports ONE int32 index per partition (multi-
index offset APs degenerate to a single lane; HW-probed), so each gather
instruction fetches 128 rows from a fused bf16 table [x(128)|pts(3)|pad]
(528B... 264B rows).  A 128-node tile needs K=15 gathers.  Descriptor
generation serializes on the Pool engine (~1.1us/instr) — that is the hard
floor.  Everything else is kept OFF the DVE<->GpSimd shared SBUF port pair
(an exclusive, full-instruction lock that stalls SWDGE desc-gen):
  * DVE two-tensor ops read one operand from PSUM (u, caggx) — no shared
    port use.
  * The attention-weighted aggregation runs on PE as 15 PSUM-accumulated
    diag(attn_k) @ g_k matmuls; diag matrices are built by the Scalar
    engine (per-partition scale of a cached identity).
  * softmax scaling and small copies run on the Scalar engine.

Sharding: nodes split contiguously over 8 cores (6250 each, padded to
6272 = 49*128); the gather table is replicated per core. No collectives.
"""

import numpy as np

N = 50000
K = 15
H = 128
NCORES = 8
SH = N // NCORES          # 6250 real nodes per core
P = 128
NT = 49                   # tiles per core
NP = NT * P               # 6272 padded nodes per core
TW = H + 4                # table row width [x 128 | pts 3 | pad]
SCALE = float(np.sqrt(64.0) + 1e-6)

_NC_CACHE = {}


def build_nc():
    import contextlib

    import concourse.bacc as bacc
    import concourse.bass as bass
    import concourse.mybir as mybir
    import concourse.tile as tile

    f32 = mybir.dt.float32
    bf16 = mybir.dt.bfloat16
    i32 = mybir.dt.int32
    Alu = mybir.AluOpType
    Act = mybir.ActivationFunctionType

    nc = bacc.Bacc("TRN2", target_bir_lowering=False, debug=False,
                   num_devices=NCORES, dynamic_dma_scratch_size=65536,
                   num_swdge_queues=1)

    tab_d = nc.dram_tensor("tab", [N, TW], bf16, kind="ExternalInput")
    xT_d = nc.dram_tensor("xT", [P, NP], f32, kind="ExternalInput")
    pts_d = nc.dram_tensor("pts", [NP, 4], f32, kind="ExternalInput")
    idx_d = nc.dram_tensor("idx", [P, NT * K], i32, kind="ExternalInput")
    Wq_d = nc.dram_tensor("Wq", [64, H], f32, kind="ExternalInput")
    Wk_d = nc.dram_tensor("Wk", [64, H], f32, kind="ExternalInput")
    bq_d = nc.dram_tensor("bq", [64, 1], f32, kind="ExternalInput")
    bk_d = nc.dram_tensor("bk", [64, 1], f32, kind="ExternalInput")
    Wc_d = nc.dram_tensor("Wc", [131, 131], f32, kind="ExternalInput")
    bc_d = nc.dram_tensor("bc", [131, 1], f32, kind="ExternalInput")
    Wo_d = nc.dram_tensor("Wo", [3, 131], f32, kind="ExternalInput")
    bo_d = nc.dram_tensor("bo", [1, 3], f32, kind="ExternalInput")
    idn_d = nc.dram_tensor("idn", [P, P], f32, kind="ExternalInput")
    out_d = nc.dram_tensor("out", [NP, 4], f32, kind="ExternalOutput")

    with tile.TileContext(nc) as tc, contextlib.ExitStack() as ctx:
        const = ctx.enter_context(tc.tile_pool(name="const", bufs=1))

        ones1 = const.tile([1, P], f32)
        nc.vector.memset(ones1[:], 1.0)

        # identity shipped as an input (keeps make_identity's gpsimd ops off
        # the Pool engine ahead of the first gather)
        idn = const.tile([P, P], f32)
        nc.scalar.dma_start(out=idn[:], in_=idn_d.ap())

        # all gather indices, preloaded once: column t*K+k holds nbr[t*128+p, k]
        idx_all = const.tile([P, NT * K], i32)
        nc.sync.dma_start(out=idx_all[:], in_=idx_d.ap())
        # mask = (nbr != 0) == min(idx, 1) for idx >= 0, built once for all
        # tiles (keeps per-tile DVE work off the shared SBUF port).
        idxf_all = const.tile([P, NT * K], f32)
        nc.vector.tensor_copy(out=idxf_all[:], in_=idx_all[:])
        mask_all = const.tile([P, NT * K], f32)
        nc.vector.tensor_scalar(out=mask_all[:], in0=idxf_all[:], scalar1=1.0,
                                scalar2=None, op0=Alu.min)

        # ---------- one-time weight prep ----------
        with tc.tile_pool(name="wprep", bufs=1) as wp:
            Wq_s = wp.tile([64, H], f32)
            nc.scalar.dma_start(out=Wq_s[:], in_=Wq_d.ap())
            Wkx_s = wp.tile([64, H + 1], f32)
            nc.scalar.dma_start(out=Wkx_s[:, 0:H], in_=Wk_d.ap())
            nc.scalar.dma_start(out=Wkx_s[:, H:H + 1], in_=bk_d.ap())
            bq_s = wp.tile([64, 1], f32)
            nc.scalar.dma_start(out=bq_s[:], in_=bq_d.ap())
            Wo_s = wp.tile([3, 131], f32)
            nc.scalar.dma_start(out=Wo_s[:], in_=Wo_d.ap())
            Wc0_s = wp.tile([P, 131], f32)
            nc.scalar.dma_start(out=Wc0_s[:], in_=Wc_d.ap()[0:P, :])
            Wc1_s = wp.tile([3, 131], f32)
            nc.scalar.dma_start(out=Wc1_s[:], in_=Wc_d.ap()[P:131, :])
            bc0_s = wp.tile([P, 1], f32)
            nc.scalar.dma_start(out=bc0_s[:], in_=bc_d.ap()[0:P, :])
            bc1_s = wp.tile([3, 1], f32)
            nc.scalar.dma_start(out=bc1_s[:], in_=bc_d.ap()[P:131, :])
            bo_s = wp.tile([1, 3], f32)
            nc.scalar.dma_start(out=bo_s[:], in_=bo_d.ap())

            with tc.tile_pool(name="wprep_psA", bufs=1, space="PSUM") as wpp:
                woT0_p = wpp.tile([P, 3], f32, space="PSUM")
                nc.tensor.transpose(out=woT0_p[:], in_=Wo_s[:, 0:P],
                                    identity=idn[0:3, 0:3])
                woT0_s = wp.tile([P, 3], f32)
                nc.vector.tensor_copy(out=woT0_s[:], in_=woT0_p[:])
                woT1_p = wpp.tile([3, 3], f32, space="PSUM")
                nc.tensor.transpose(out=woT1_p[:], in_=Wo_s[:, P:131],
                                    identity=idn[0:3, 0:3])
                woT1_s = wp.tile([3, 3], f32)
                nc.vector.tensor_copy(out=woT1_s[:], in_=woT1_p[:])

                # Wf = Wo @ Wc  [3,131]
                Wf_p = wpp.tile([3, 131], f32, space="PSUM")
                nc.tensor.matmul(out=Wf_p[:], lhsT=woT0_s[:], rhs=Wc0_s[:],
                                 start=True, stop=False)
                nc.tensor.matmul(out=Wf_p[:], lhsT=woT1_s[:], rhs=Wc1_s[:],
                                 start=False, stop=True)
                Wf_s = wp.tile([3, 131], f32)
                nc.vector.tensor_copy(out=Wf_s[:], in_=Wf_p[:])

                # bfT = (Wo @ bc).T [1,3] ; + bo
                bfT_p = wpp.tile([1, 3], f32, space="PSUM")
                nc.tensor.matmul(out=bfT_p[:], lhsT=bc0_s[:], rhs=woT0_s[:],
                                 start=True, stop=False)
                nc.tensor.matmul(out=bfT_p[:], lhsT=bc1_s[:], rhs=woT1_s[:],
                                 start=False, stop=True)
                bfT_s = wp.tile([1, 3], f32)
                nc.vector.tensor_add(out=bfT_s[:], in0=bfT_p[:], in1=bo_s[:])

            with tc.tile_pool(name="wprep_psB", bufs=1, space="PSUM") as wpp2:
                # wfx rows: [Wf[c,0:131] | 0 | I3[c,:]-Wf[c,128:131] | bf[c]],
                # replicated across partitions (stride-0 partition APs are
                # illegal on DVE): e_c row extract + ones-outer-product.
                # The bias column pairs with caggx[:,135] = sum(exp), so the
                # whole output is one matvec scaled by 1/sum(exp) at the end.
                # (no memset: every [:, c, :] slice is fully overwritten by
                # the Act copy from wfx_p below before any read)
                wfx_s = const.tile([P, 3, 136], f32)
                for c in range(3):
                    row_p = wpp2.tile([1, 131], f32, space="PSUM",
                                      name="row_p")
                    nc.tensor.matmul(out=row_p[:], lhsT=idn[0:3, c:c + 1],
                                     rhs=Wf_s[:], start=True, stop=True)
                    row_s = wp.tile([1, 136], f32, name=f"row_s{c}")
                    nc.vector.memset(row_s[:], 0.0)
                    nc.scalar.activation(out=row_s[:, 0:131], in_=row_p[:],
                                         func=Act.Copy)
                    # cols 132..134: I3[c,:] - Wf[c,128:131]
                    rowI = wp.tile([1, 3], f32, name=f"rowI{c}")
                    nc.vector.memset(rowI[:], 0.0)
                    nc.vector.memset(rowI[:, c:c + 1], 1.0)
                    nc.vector.tensor_sub(out=row_s[:, 132:135],
                                         in0=rowI[:],
                                         in1=row_s[:, 128:131])
                    # col 135: bf[c]
                    nc.scalar.activation(out=row_s[:, 135:136],
                                         in_=bfT_s[:, c:c + 1],
                                         func=Act.Copy)
                    wfx_p = wpp2.tile([P, 136], f32, space="PSUM",
                                      name="wfx_p")
                    nc.tensor.matmul(out=wfx_p[:], lhsT=ones1[:],
                                     rhs=row_s[:], start=True, stop=True)
                    nc.scalar.activation(out=wfx_s[:, c, :], in_=wfx_p[:],
                                         func=Act.Copy)

                # M_ext = [Wq.T @ Wk | Wq.T @ bk] / scale  [128, 129]
                Mw_p = wpp2.tile([P, H + 1], f32, space="PSUM")
                nc.tensor.matmul(out=Mw_p[:], lhsT=Wq_s[:], rhs=Wkx_s[:],
                                 start=True, stop=True)
                Mx_s = const.tile([P, H + 1], f32)
                nc.scalar.activation(out=Mx_s[:], in_=Mw_p[:], func=Act.Copy,
                                     scale=1.0 / SCALE)

                # [c1 | s2] = [bq @ Wk | bq.bk] / scale  [1, 129]
                cs_p = wpp2.tile([1, H + 1], f32, space="PSUM")
                nc.tensor.matmul(out=cs_p[:], lhsT=bq_s[:], rhs=Wkx_s[:],
                                 start=True, stop=True)
                cs_s = const.tile([1, H + 1], f32)
                nc.scalar.activation(out=cs_s[:], in_=cs_p[:], func=Act.Copy,
                                     scale=1.0 / SCALE)

        # ---------- main loop ----------
        sb = ctx.enter_context(tc.tile_pool(name="sb", bufs=4))
        gp = ctx.enter_context(tc.tile_pool(name="gp", bufs=8))
        big = ctx.enter_context(tc.tile_pool(name="big", bufs=4))
        dg = ctx.enter_context(tc.tile_pool(name="dg", bufs=4))
        pp = ctx.enter_context(tc.tile_pool(name="pp", bufs=4, space="PSUM"))

        for t in range(NT):
            rows = slice(t * P, (t + 1) * P)

            xT_t = sb.tile([P, P], f32)
            nc.sync.dma_start(out=xT_t[:], in_=xT_d.ap()[:, rows])
            idx_t = idx_all[:, t * K:(t + 1) * K]
            pts_t = sb.tile([P, 4], f32)
            nc.sync.dma_start(out=pts_t[:], in_=pts_d.ap()[rows, :])

            # K single-index-per-partition indirect gathers (128 rows each),
            # spread over the 4 SWDGE queues for descriptor-ring headroom.
            g_t = gp.tile([P, K, TW], bf16)
            for k in range(K):
                nc.gpsimd.indirect_dma_start(
                    out=g_t[:, k, :],
                    out_offset=None,
                    in_=tab_d.ap(),
                    in_offset=bass.IndirectOffsetOnAxis(
                        ap=idx_t[:, k:k + 1], axis=0),
                )

            # u_ext = xT.T @ M_ext + bcast([c1|s2])  ->  [p, 129] = [u | s]
            # (kept in PSUM: DVE reads of it avoid the shared SBUF port)
            u_p = pp.tile([P, H + 1], f32, space="PSUM")
            nc.tensor.matmul(out=u_p[:], lhsT=xT_t[:], rhs=Mx_s[:],
                             start=True, stop=False)
            nc.tensor.matmul(out=u_p[:], lhsT=ones1[:], rhs=cs_s[:],
                             start=False, stop=True)
            s_f = sb.tile([P, 1], f32)
            nc.scalar.activation(out=s_f[:], in_=u_p[:, H:H + 1],
                                 func=Act.Copy)

            # scores: raw[p,k] = sum_h u[p,h] * g[p,k,h]  (+ s, * mask)
            # (split so only the last 3 slots' product trails the final
            # gather — shortens the per-tile critical chain)
            prod = big.tile([P, K, H], bf16)
            nc.vector.tensor_mul(
                out=prod[:, 0:12, :],
                in0=g_t[:][:, 0:12, 0:H],
                in1=u_p[:, 0:H].unsqueeze(1).to_broadcast([P, 12, H]),
            )
            nc.vector.tensor_mul(
                out=prod[:, 12:K, :],
                in0=g_t[:][:, 12:K, 0:H],
                in1=u_p[:, 0:H].unsqueeze(1).to_broadcast([P, K - 12, H]),
            )
            raw = sb.tile([P, K], f32)
            nc.vector.tensor_reduce(out=raw[:, 0:12], in_=prod[:, 0:12, :],
                                    axis=mybir.AxisListType.X, op=Alu.add)
            nc.vector.tensor_reduce(out=raw[:, 12:K], in_=prod[:, 12:K, :],
                                    axis=mybir.AxisListType.X, op=Alu.add)
            # scores + softmax numerators, split 12+3 like the product so
            # the first 12 diag/matmul aggregation steps run before the
            # tile's last gather lands.  The normalization 1/sum(exp) is
            # applied once at the very end (aggregation uses raw exp
            # weights), with sum(exp) = se_a + se_b.
            sc = sb.tile([P, K], f32)
            scm = sb.tile([P, K], f32)
            e_t = sb.tile([P, K], f32)
            se_a = sb.tile([P, 1], f32)
            se_b = sb.tile([P, 1], f32)
            nc.vector.tensor_scalar(out=sc[:, 0:12], in0=raw[:, 0:12],
                                    scalar1=s_f[:], scalar2=None,
                                    op0=Alu.add)
            nc.vector.tensor_mul(out=scm[:, 0:12], in0=sc[:, 0:12],
                                 in1=mask_all[:, t * K:t * K + 12])
            nc.scalar.activation(out=e_t[:, 0:12], in_=scm[:, 0:12],
                                 func=Act.Exp, bias=0.0, scale=1.0,
                                 accum_out=se_a[:])
            nc.vector.tensor_scalar(out=sc[:, 12:K], in0=raw[:, 12:K],
                                    scalar1=s_f[:], scalar2=None,
                                    op0=Alu.add)
            nc.vector.tensor_mul(out=scm[:, 12:K], in0=sc[:, 12:K],
                                 in1=mask_all[:, t * K + 12:(t + 1) * K])
            nc.scalar.activation(out=e_t[:, 12:K], in_=scm[:, 12:K],
                                 func=Act.Exp, bias=0.0, scale=1.0,
                                 accum_out=se_b[:])
            se_t = sb.tile([P, 1], f32)
            nc.vector.tensor_add(out=se_t[:], in0=se_a[:], in1=se_b[:])
            r_t = sb.tile([P, 1], f32)
            nc.vector.reciprocal(out=r_t[:], in_=se_t[:])

            # caggx = [sum_k e_k * g_k (132) | pts*S (3) | S],  S = sum(exp),
            # accumulated on PE: caggx += diag(e_k) @ g_k.  diag built on
            # the Scalar engine (per-partition scale of identity).
            caggx_p = pp.tile([P, 136], f32, space="PSUM")
            for k in range(K):
                dg_t = dg.tile([P, P], bf16)
                nc.scalar.activation(out=dg_t[:], in_=idn[:], func=Act.Copy,
                                     scale=e_t[:, k:k + 1])
                nc.tensor.matmul(out=caggx_p[:, 0:132], lhsT=dg_t[:],
                                 rhs=g_t[:, k, :], start=(k == 0),
                                 stop=(k == K - 1))
            nc.scalar.activation(out=caggx_p[:, 132:135], in_=pts_t[:, 0:3],
                                 func=Act.Copy, scale=se_t[:])
            nc.scalar.activation(out=caggx_p[:, 135:136], in_=se_t[:],
                                 func=Act.Copy)

            # disp[p,c] = Wfx[c,:] . caggx[p,:]  (PSUM-side mul avoids the
            # DVE<->GpSimd shared SBUF port); out = disp / S
            disp = sb.tile([P, 3], f32)
            junk = sb.tile([P, 136], f32)
            for c in range(3):
                nc.vector.tensor_mul(out=junk[:], in0=caggx_p[:, 0:136],
                                     in1=wfx_s[:, c, :])
                nc.vector.tensor_reduce(out=disp[:, c:c + 1], in_=junk[:],
                                        axis=mybir.AxisListType.X,
                                        op=Alu.add)

            out_t = sb.tile([P, 4], f32)
            nc.scalar.activation(out=out_t[:, 0:3], in_=disp[:],
                                 func=Act.Copy, scale=r_t[:])
            nc.sync.dma_start(out=out_d.ap()[rows, 0:3], in_=out_t[:, 0:3])

    nc.compile()
    return nc


def get_nc():
    if "nc" not in _NC_CACHE:
        _NC_CACHE["nc"] = build_nc()
    return _NC_CACHE["nc"]


def make_in_maps(sampled_points, sampled_x, Wq, bq, Wk, bk, Wc, bc, Wo, bo,
                 edge_index_filtered):
    sampled_points = np.ascontiguousarray(sampled_points, np.float32)
    sampled_x = np.ascontiguousarray(sampled_x, np.float32)
    nbr = np.ascontiguousarray(
        np.asarray(edge_index_filtered)[1].reshape(N, K)).astype(np.int32)

    import ml_dtypes
    tab = np.zeros((N, TW), ml_dtypes.bfloat16)
    tab[:, :H] = sampled_x.astype(ml_dtypes.bfloat16)
    tab[:, H:H + 3] = sampled_points.astype(ml_dtypes.bfloat16)

    shared = {
        "tab": tab,
        "idn": np.eye(P, dtype=np.float32),
        "Wq": np.ascontiguousarray(Wq, np.float32),
        "Wk": np.ascontiguousarray(Wk, np.float32),
        "bq": np.ascontiguousarray(np.reshape(bq, (64, 1)), np.float32),
        "bk": np.ascontiguousarray(np.reshape(bk, (64, 1)), np.float32),
        "Wc": np.ascontiguousarray(Wc, np.float32),
        "bc": np.ascontiguousarray(np.reshape(bc, (131, 1)), np.float32),
        "Wo": np.ascontiguousarray(Wo, np.float32),
        "bo": np.ascontiguousarray(np.reshape(bo, (1, 3)), np.float32),
    }

    in_maps = []
    for c in range(NCORES):
        rows = slice(c * SH, (c + 1) * SH)
        xT = np.zeros((P, NP), np.float32)
        xT[:, :SH] = sampled_x[rows].T
        pts4 = np.zeros((NP, 4), np.float32)
        pts4[:SH, :3] = sampled_points[rows]
        nb = np.zeros((NP, K), np.int32)
        nb[:SH] = nbr[rows]
        # [P, NT*K]: column t*K+k = nbr[t*128+p, k]
        idx = np.ascontiguousarray(
            nb.reshape(NT, P, K).transpose(1, 0, 2).reshape(P, NT * K))
        in_maps.append({**shared, "xT": xT, "pts": pts4, "idx": idx})
    return in_maps


def unshard(results):
    out = np.concatenate(
        [results[c]["out"][:SH, :3] for c in range(NCORES)], axis=0)
    return np.ascontiguousarray(out)


def kernel(**inputs):
    from concourse.bass_utils import run_bass_kernel_spmd

    nc = get_nc()
    in_maps = make_in_maps(**inputs)
    res = run_bass_kernel_spmd(nc, in_maps, core_ids=list(range(NCORES)))
    return unshard(res.results)
